# revision 37
# baseline (speedup 1.0000x reference)
"""AttentionRNN Trainium2 kernel — hybrid dual-port recurrence.

Problem: B=128, T=512, H=1024, V=128
  xe = Wxh[x]                               (gather == onehot(x) @ Wxh)
  h_t = tanh(xe_t + h_{t-1} @ Whh + bh)     (512 sequential steps)
  S   = Hs @ Hs^T  (per batch);  W = softmax(S, axis=-1)
  ctx = W @ Hs;    out = [Hs, ctx] @ fc_w.T + fc_b

Sharding: data-parallel over batch, 16 batches per core, 8 cores. Params
replicated. No collectives.

Recurrence design (the bottleneck): per step ~1.15 M Whh elements must
enter the PE array. The array has TWO independent SBUF read ports:
 - moving-operand port (~128 elem/cycle @ 2.4 GHz)
 - weight port (FWL, ~2 bf16 elem/cycle/partition @ 1.2 GHz)
A single formulation uses only one port. We split the output hidden dim:
 - part-a (j in [0,512)): h-stationary. lhsT = h chunk [128,16] (cheap
   16-col weight loads), Whh streams as the MOVING operand. 4 PE column
   groups (tile_position=(0,32g)), group g computes j-chunk g, N=128.
   psum [32g+b, c] = z[b, 128g+c] -> grouped layout, tanh -> ONE
   [128,128] PE transpose gives hidden-major h for chunks 0-3.
 - part-b (j in [512,1024)): Whh-stationary z^T form. lhsT = Whh chunk
   [128,128] (flows through the WEIGHT port via fast-weight-load,
   overlapped with part-a's streaming), rhs = h chunk [128,16], N=16.
   psum [p, 16jj+b] = z[b, 512+128jj+p] -> hidden-major DIRECTLY (no
   transpose), tanh writes ht chunks 4-7.
Both ports run concurrently -> ~2x the single-port floor. The onehot
vocab contribution rides in both parts (host-precomputed onehot in SBUF).
State ht ping-pongs [128, (k8 b16)] bf16 hidden-major; each step appends
h_t to HsT (bf16) for the attention phase off the critical path.

Attention (per batch): exp WITHOUT max-subtraction (P = exp(S) symmetric),
ctx@fc_wc.T = P @ (Hs@fc_wc.T) = P @ G associativity, P^T blocks read from
P via symmetry, rowsum normalization via per-partition DVE scale. All
matmul operands bf16 (fp32r pays 4 cycles/row at N<256).
"""

import os
import sys

sys.path.insert(0, "/opt/trn_rl_repo")

import ml_dtypes
import numpy as np

import concourse.bass as bass
import concourse.bacc as bacc
import concourse.mybir as mybir
import concourse.tile as tile
from concourse.bass_utils import run_bass_kernel_spmd
from concourse.masks import make_identity

B, T, H, V = 128, 512, 1024, 128
NCORES = 8
BS = B // NCORES  # 16 batches per core
KCH = H // 128  # 8 hidden chunks
F32 = mybir.dt.float32
BF16 = mybir.dt.bfloat16
AF = mybir.ActivationFunctionType
BFNP = ml_dtypes.bfloat16

UNROLL = 32


def build_nc(t_steps=T):
    nc = bacc.Bacc(None, target_bir_lowering=False)

    # ---- DRAM I/O (all weights host-prepped bf16) ----
    whh_a_d = nc.dram_tensor("whh_a", [128, KCH * 512], BF16, kind="ExternalInput")
    whh_b_d = nc.dram_tensor("whh_b", [128, 4 * KCH * 128], BF16, kind="ExternalInput")
    wxh_a_d = nc.dram_tensor("wxh_a", [V, 512], BF16, kind="ExternalInput")
    wxh_b_d = nc.dram_tensor("wxh_b", [V, 512], BF16, kind="ExternalInput")
    oh_d = nc.dram_tensor("oh", [V, (t_steps + UNROLL) * BS], BF16, kind="ExternalInput")
    fcw2_d = nc.dram_tensor("fcw2", [128, 16 * V], BF16, kind="ExternalInput")
    fcb_d = nc.dram_tensor("fcb", [1, V], BF16, kind="ExternalInput")
    out_d = nc.dram_tensor("out", [BS, t_steps, V], F32, kind="ExternalOutput")
    hdbg_d = None
    if os.environ.get("HDBG", "0") == "1":
        hdbg_d = nc.dram_tensor(
            "hdbg", [128, KCH * BS * t_steps], BF16, kind="ExternalOutput"
        )

    with tile.TileContext(nc) as tc:
        with tc.tile_pool(name="persist", bufs=1) as pp:
            hst = pp.tile([128, KCH * BS * t_steps], BF16, tag="hst")
            whh_a = pp.tile([128, KCH * 512], BF16, tag="whh_a")
            whh_b = pp.tile([128, 4 * KCH * 128], BF16, tag="whh_b")
            wxh_a = pp.tile([128, 512], BF16, tag="wxh_a")
            wxh_b = pp.tile([128, 512], BF16, tag="wxh_b")
            oh = pp.tile([128, (t_steps + UNROLL) * BS], BF16, tag="oh")
            ohwin = pp.tile([128, UNROLL * BS + 112], BF16, tag="ohwin")
            fcw2_sb = pp.tile([128, 16 * V], BF16, tag="fcw2")
            fcb_row = pp.tile([1, V], BF16, tag="fcb")
            id_bf = pp.tile([128, 128], BF16, tag="ident")
            # 112 slack cols so M=128-padded lhsT slices [16k, 16k+128) stay
            # in-bounds (junk output partitions are ignored)
            hta = pp.tile([128, 240], BF16, tag="hta")
            htb = pp.tile([128, 240], BF16, tag="htb")
            hga = pp.tile([128, 512], BF16, tag="hga")
            hgb = pp.tile([128, 512], BF16, tag="hgb")
            ones_bf = pp.tile([1, 128], BF16, tag="onesb")

            nc.gpsimd.memset(ones_bf[:], 1.0)
            nc.gpsimd.memset(hta[:], 0.0)  # h_{-1} = 0 (+ slack cols stay 0)
            nc.gpsimd.memset(htb[:], 0.0)
            nc.gpsimd.memset(hga[:], 0.0)  # rows 16-127 stay 0 forever
            nc.gpsimd.memset(hgb[:], 0.0)
            nc.gpsimd.memset(ohwin[:], 0.0)  # slack cols stay 0

            nc.gpsimd.dma_start(whh_a[:], whh_a_d[:])
            nc.gpsimd.dma_start(whh_b[:], whh_b_d[:])
            nc.gpsimd.dma_start(wxh_a[:], wxh_a_d[:])
            nc.gpsimd.dma_start(wxh_b[:], wxh_b_d[:])
            nc.gpsimd.dma_start(oh[:], oh_d[:])
            nc.gpsimd.dma_start(fcw2_sb[:], fcw2_d[:])
            nc.gpsimd.dma_start(fcb_row[:], fcb_d[:])

            with tc.tile_pool(name="idtmp", bufs=1) as it_:
                id_f = it_.tile([128, 128], F32, tag="identf")
                make_identity(nc, id_f[:])
                nc.vector.tensor_copy(id_bf[:], id_f[:])

            hst_v = hst.rearrange("p (kb t) -> p kb t", t=t_steps)
            oh_w = oh.rearrange("p (t b) -> p t b", b=BS)
            ohwin_w = ohwin.rearrange("p (s b) -> p s b", b=BS)

            # ---- recurrence ----
            with (
                tc.tile_pool(name="psa", bufs=2, space="PSUM") as pap,
                tc.tile_pool(name="psb", bufs=2, space="PSUM") as pbp,
                tc.tile_pool(name="pst", bufs=2, space="PSUM") as ptp,
            ):
                def step(t_expr, parity, s=None, iv=None):
                    ht_cur = hta if parity == 0 else htb
                    ht_new = htb if parity == 0 else hta
                    h_grp = hga if parity == 0 else hgb
                    if s is None:
                        s = t_expr % UNROLL  # python-unrolled path
                    oh_t = ohwin_w[:, s, :]  # [128, 16] fixed address

                    pa = pap.tile([128, 512], F32, tag="pa", name="pa")
                    pb = pbp.tile([128, 64], F32, tag="pb", name="pb")

                    # Every matmul is (128,128)-mode (K=128, M=128): mode
                    # transitions drain the PE pipeline, so part-a pads its
                    # stationary to M=128 (output partitions 16-127 are junk
                    # and ignored). The weight port is the limiter
                    # (~54ns/128-col FWL load); part-a streams Whh through
                    # the moving port instead (1 load + N=512 stream per k).
                    # k=-1 is the vocab (onehot) contribution; chunks 4-7
                    # first (their h is ready earliest).
                    for k in (-1, 4, 5, 6, 7, 0, 1, 2, 3):
                        first, last = k == -1, k == 3
                        if k < 0:
                            lb = [wxh_b[:, 128 * jj : 128 * jj + 128] for jj in range(4)]
                            la = ohwin[:, 16 * s : 16 * s + 128]
                            ra = wxh_a[:, :]
                            rb = oh_t
                        else:
                            lb = [
                                whh_b[:, (jj * KCH + k) * 128 : (jj * KCH + k) * 128 + 128]
                                for jj in range(4)
                            ]
                            la = ht_cur[:, 16 * k : 16 * k + 128]
                            ra = whh_a[:, k * 512 : k * 512 + 512]
                            rb = ht_cur[:, 16 * k : 16 * k + 16]
                        # on the last k, emit all part-b matmuls before
                        # part-a's so pb's accumulation stops ~213ns earlier
                        # (ACT_b gates the next step's first hidden matmuls)
                        nsplit = 4 if last else 2
                        for jj in range(nsplit):
                            nc.tensor.matmul(
                                pb[:, 16 * jj : 16 * jj + 16],
                                lb[jj],
                                rb,
                                start=(first and jj == 0), stop=last,
                                skip_group_check=True,
                            )
                        nc.tensor.matmul(
                            pa[:, :], la, ra,
                            start=first, stop=last,
                            skip_group_check=True,
                        )
                        for jj in range(nsplit, 4):
                            nc.tensor.matmul(
                                pb[:, 16 * jj : 16 * jj + 16],
                                lb[jj],
                                rb,
                                start=False, stop=last,
                                skip_group_check=True,
                            )
                        if first and s == UNROLL - 1 and iv is not None:
                            # prefetch next onehot window (single buffer; WAR
                            # on this step's vocab matmuls keeps semantics)
                            nc.vector.tensor_copy(
                                ohwin_w[:, 0:UNROLL, :],
                                oh_w[:, bass.DynSlice(iv + UNROLL, UNROLL), :],
                            )
                    # tanh: part-b lands hidden-major directly in ht chunks
                    # 4-7; part-a batch-major into h_grp rows 0-15 (2 halves
                    # so transposes can start after the first)
                    nc.scalar.activation(ht_new[:, 64:128], pb[:, :], AF.Tanh)
                    nc.scalar.activation(h_grp[0:16, 0:256], pa[0:16, 0:256], AF.Tanh)
                    nc.scalar.activation(h_grp[0:16, 256:512], pa[0:16, 256:512], AF.Tanh)
                    # part-a -> hidden-major chunks 0-3 via identity-rhs
                    # matmuls, still (128,128) mode: out[p,b] =
                    # sum_r h_grp[r, 128k+p] * I[r, b] = h[b, 128k+p]
                    pt = ptp.tile([128, 64], F32, tag="pt", name="pt")
                    for kk in range(4):
                        nc.tensor.matmul(
                            pt[:, 16 * kk : 16 * kk + 16],
                            h_grp[:, 128 * kk : 128 * kk + 128],
                            id_bf[:, 0:16],
                            start=(kk == 0), stop=(kk == 3),
                            skip_group_check=True,
                        )
                    nc.vector.tensor_copy(ht_new[:, 0:64], pt[:, :])
                    # append h_t to HsT (off critical path)
                    nc.vector.tensor_copy(
                        hst_v[:, :, bass.ts(t_expr, 1)],
                        ht_new.rearrange("p (kb one) -> p kb one", one=1)[:, 0:128, :],
                    )

                if t_steps <= 32:
                    for t in range(t_steps):
                        if t % UNROLL == 0:
                            nc.vector.tensor_copy(
                                ohwin_w[:, 0:UNROLL, :],
                                oh_w[:, t : t + UNROLL, :],
                            )
                        step(t, t % 2)
                else:
                    assert t_steps % UNROLL == 0
                    nc.vector.tensor_copy(
                        ohwin_w[:, 0:UNROLL, :], oh_w[:, 0:UNROLL, :]
                    )
                    with tc.For_i(
                        0, t_steps, UNROLL,
                        hint_engines=(mybir.EngineType.PE,),
                        staggered_reset=True,
                    ) as iv:
                        for s in range(UNROLL):
                            step(iv + s, s % 2, s=s, iv=iv)

            if hdbg_d is not None:
                nc.sync.dma_start(hdbg_d[:, :], hst[:, :])

            # ---- attention + fc, per batch ----
            with (
                tc.tile_pool(name="attn", bufs=1) as ap_,
                tc.tile_pool(name="attn2", bufs=2) as ap2,
                tc.tile_pool(name="psS", bufs=2, space="PSUM") as psS_p,
                tc.tile_pool(name="psF", bufs=2, space="PSUM") as psF_p,
                tc.tile_pool(name="ps2", bufs=2, space="PSUM") as ps2_p,
            ):
                n_tc = t_steps // 128  # t-chunks of 128
                for b in range(BS):
                    def hs(k, sl):  # HsT tile for (k-chunk, slice of t)
                        return hst_v[:, k * BS + b, sl]

                    p_sb = ap_.tile([128, n_tc * t_steps], BF16, tag="p_sb")
                    rinv = ap_.tile([128, n_tc], F32, tag="rinv")
                    g_sb = ap_.tile([128, n_tc * V], BF16, tag="g_sb")
                    # Fused pass per t-chunk: scores (N=512) and
                    # [out1 | G] (N=256, fcw halves fused) share each
                    # hs(k, c) stationary load. psF[:, 0:128] = Hs@fc_wh.T
                    # (+bias), psF[:, 128:256] = G chunk = Hs@fc_wc.T.
                    o1_sb = ap_.tile([128, n_tc * V], F32, tag="o1_sb")
                    for c in range(n_tc):
                        psS = psS_p.tile([128, t_steps], F32, tag="psS")
                        psF = psF_p.tile([128, 2 * V], F32, tag="psF", name="psF")
                        for k in range(KCH):
                            nc.tensor.matmul(
                                psS[:],
                                hs(k, slice(128 * c, 128 * c + 128)),
                                hs(k, slice(0, t_steps)),
                                start=(k == 0),
                                stop=(k == KCH - 1),
                            )
                            nc.tensor.matmul(
                                psF[:],
                                hs(k, slice(128 * c, 128 * c + 128)),
                                fcw2_sb[:, 256 * k : 256 * k + 256],
                                start=(k == 0),
                                stop=False,
                                skip_group_check=True,
                            )
                        nc.tensor.matmul(
                            psF[:, 0:V],
                            ones_bf[:],
                            fcb_row[:],
                            start=False,
                            stop=True,
                            skip_group_check=True,
                        )
                        rowsum = ap2.tile([128, 1], F32, tag="rowsum")
                        nc.scalar.activation(
                            p_sb[:, c * t_steps : (c + 1) * t_steps],
                            psS[:],
                            AF.Exp,
                            accum_out=rowsum[:],
                        )
                        nc.vector.reciprocal(rinv[:, c : c + 1], rowsum[:])
                        nc.vector.tensor_copy(
                            g_sb[:, c * V : (c + 1) * V], psF[:, V : 2 * V]
                        )
                        nc.vector.tensor_copy(
                            o1_sb[:, c * V : (c + 1) * V], psF[:, 0:V]
                        )
                    # out[t-chunk c] = psF[0:V] + rinv * (P @ G)
                    for c in range(n_tc):
                        ps2 = ps2_p.tile([128, V], F32, tag="ps2")
                        for i in range(n_tc):
                            # lhsT = P^T block (i,c) == P block, by symmetry
                            nc.tensor.matmul(
                                ps2[:],
                                p_sb[:, i * t_steps + 128 * c : i * t_steps + 128 * c + 128],
                                g_sb[:, i * V : (i + 1) * V],
                                start=(i == 0),
                                stop=(i == n_tc - 1),
                            )
                        o2 = ap2.tile([128, V], F32, tag="o2")
                        nc.vector.tensor_scalar_mul(o2[:], ps2[:], rinv[:, c : c + 1])
                        oo = ap2.tile([128, V], F32, tag="oo")
                        nc.vector.tensor_add(oo[:], o1_sb[:, c * V : (c + 1) * V], o2[:])
                        nc.sync.dma_start(out_d[b, 128 * c : 128 * c + 128, :], oo[:])

    nc.compile()
    return nc


def _prep_core_inputs(inputs, core, t_steps=T):
    x = np.asarray(inputs["x"])[core * BS : (core + 1) * BS, :t_steps]
    wxhb = (
        np.asarray(inputs["Wxh"]).astype(np.float32)
        + np.asarray(inputs["bh"]).astype(np.float32)[None, :]
    )
    whh = np.asarray(inputs["Whh"]).astype(np.float32)
    w4 = whh.reshape(KCH, 128, KCH, 128)
    # whh_a[p, (k, g, j)] = Whh[128k+p, 128g+j], g < 4
    whh_a = np.ascontiguousarray(w4[:, :, :4, :].transpose(1, 0, 2, 3)).reshape(128, -1)
    # whh_b[p, (jj, k, j)] = Whh[128k+p, 512 + 128jj + j]
    whh_b = np.ascontiguousarray(w4[:, :, 4:, :].transpose(1, 2, 0, 3)).reshape(128, -1)
    # onehot: oh[v, 16t + b] = (x[b, t] == v); one zero window of padding
    oh = np.zeros((V, (t_steps + UNROLL) * BS), dtype=BFNP)
    oh[x.T.reshape(-1), np.arange(t_steps * BS)] = 1
    return {
        "whh_a": whh_a.astype(BFNP),
        "whh_b": whh_b.astype(BFNP),
        "wxh_a": np.ascontiguousarray(wxhb[:, :512]).astype(BFNP),
        "wxh_b": np.ascontiguousarray(wxhb[:, 512:]).astype(BFNP),
        "oh": oh,
        # fcw2[p, k*256 + half*128 + v] = fc_w[v, half*1024 + 128k + p]
        "fcw2": np.ascontiguousarray(
            np.asarray(inputs["fc_w"])
            .astype(np.float32)
            .T.reshape(2, KCH, 128, V)
            .transpose(2, 1, 0, 3)
        ).reshape(128, 16 * V).astype(BFNP),
        "fcb": np.asarray(inputs["fc_b"]).astype(np.float32).reshape(1, V).astype(BFNP),
    }


def kernel(x, Wxh, Whh, bh, fc_w, fc_b, t_steps=T, trace=False):
    inputs = dict(x=x, Wxh=Wxh, Whh=Whh, bh=bh, fc_w=fc_w, fc_b=fc_b)
    nc = build_nc(t_steps)
    in_maps = [_prep_core_inputs(inputs, c, t_steps) for c in range(NCORES)]
    res = run_bass_kernel_spmd(nc, in_maps, core_ids=list(range(NCORES)), trace=trace)
    out = np.concatenate([r["out"] for r in res.results], axis=0)
    if trace:
        print(f"HW exec time: {res.exec_time_ns} ns", file=sys.stderr)
    return out


# revision 38
# speedup vs baseline: 1.0145x; 1.0145x over previous
"""AttentionRNN Trainium2 kernel — hybrid dual-port recurrence.

Problem: B=128, T=512, H=1024, V=128
  xe = Wxh[x]                               (gather == onehot(x) @ Wxh)
  h_t = tanh(xe_t + h_{t-1} @ Whh + bh)     (512 sequential steps)
  S   = Hs @ Hs^T  (per batch);  W = softmax(S, axis=-1)
  ctx = W @ Hs;    out = [Hs, ctx] @ fc_w.T + fc_b

Sharding: data-parallel over batch, 16 batches per core, 8 cores. Params
replicated. No collectives.

Recurrence design (the bottleneck): per step ~1.15 M Whh elements must
enter the PE array. The array has TWO independent SBUF read ports:
 - moving-operand port (~128 elem/cycle @ 2.4 GHz)
 - weight port (FWL, ~2 bf16 elem/cycle/partition @ 1.2 GHz)
A single formulation uses only one port. We split the output hidden dim:
 - part-a (j in [0,512)): h-stationary. lhsT = h chunk [128,16] (cheap
   16-col weight loads), Whh streams as the MOVING operand. 4 PE column
   groups (tile_position=(0,32g)), group g computes j-chunk g, N=128.
   psum [32g+b, c] = z[b, 128g+c] -> grouped layout, tanh -> ONE
   [128,128] PE transpose gives hidden-major h for chunks 0-3.
 - part-b (j in [512,1024)): Whh-stationary z^T form. lhsT = Whh chunk
   [128,128] (flows through the WEIGHT port via fast-weight-load,
   overlapped with part-a's streaming), rhs = h chunk [128,16], N=16.
   psum [p, 16jj+b] = z[b, 512+128jj+p] -> hidden-major DIRECTLY (no
   transpose), tanh writes ht chunks 4-7.
Both ports run concurrently -> ~2x the single-port floor. The onehot
vocab contribution rides in both parts (host-precomputed onehot in SBUF).
State ht ping-pongs [128, (k8 b16)] bf16 hidden-major; each step appends
h_t to HsT (bf16) for the attention phase off the critical path.

Attention (per batch): exp WITHOUT max-subtraction (P = exp(S) symmetric),
ctx@fc_wc.T = P @ (Hs@fc_wc.T) = P @ G associativity, P^T blocks read from
P via symmetry, rowsum normalization via per-partition DVE scale. All
matmul operands bf16 (fp32r pays 4 cycles/row at N<256).
"""

import os
import sys

sys.path.insert(0, "/opt/trn_rl_repo")

import ml_dtypes
import numpy as np

import concourse.bass as bass
import concourse.bacc as bacc
import concourse.mybir as mybir
import concourse.tile as tile
from concourse.bass_utils import run_bass_kernel_spmd
from concourse.masks import make_identity

B, T, H, V = 128, 512, 1024, 128
NCORES = 8
BS = B // NCORES  # 16 batches per core
KCH = H // 128  # 8 hidden chunks
F32 = mybir.dt.float32
BF16 = mybir.dt.bfloat16
AF = mybir.ActivationFunctionType
BFNP = ml_dtypes.bfloat16

UNROLL = 32


def build_nc(t_steps=T):
    nc = bacc.Bacc(None, target_bir_lowering=False)

    # ---- DRAM I/O (all weights host-prepped bf16) ----
    whh_a_d = nc.dram_tensor("whh_a", [128, KCH * 512], BF16, kind="ExternalInput")
    whh_b_d = nc.dram_tensor("whh_b", [128, 4 * KCH * 128], BF16, kind="ExternalInput")
    wxh_a_d = nc.dram_tensor("wxh_a", [V, 512], BF16, kind="ExternalInput")
    wxh_b_d = nc.dram_tensor("wxh_b", [V, 512], BF16, kind="ExternalInput")
    oh_d = nc.dram_tensor("oh", [V, (t_steps + UNROLL) * BS], BF16, kind="ExternalInput")
    fcw2_d = nc.dram_tensor("fcw2", [128, 16 * V], BF16, kind="ExternalInput")
    fcb_d = nc.dram_tensor("fcb", [1, V], BF16, kind="ExternalInput")
    out_d = nc.dram_tensor("out", [BS, t_steps, V], F32, kind="ExternalOutput")
    hdbg_d = None
    if os.environ.get("HDBG", "0") == "1":
        hdbg_d = nc.dram_tensor(
            "hdbg", [128, KCH * BS * t_steps], BF16, kind="ExternalOutput"
        )

    with tile.TileContext(nc) as tc:
        with tc.tile_pool(name="persist", bufs=1) as pp:
            hst = pp.tile([128, KCH * BS * t_steps], BF16, tag="hst")
            whh_a = pp.tile([128, KCH * 512], BF16, tag="whh_a")
            whh_b = pp.tile([128, 4 * KCH * 128], BF16, tag="whh_b")
            wxh_a = pp.tile([128, 512], BF16, tag="wxh_a")
            wxh_b = pp.tile([128, 512], BF16, tag="wxh_b")
            oh = pp.tile([128, (t_steps + UNROLL) * BS], BF16, tag="oh")
            ohwin = pp.tile([128, UNROLL * BS + 112], BF16, tag="ohwin")
            fcw2_sb = pp.tile([128, 16 * V], BF16, tag="fcw2")
            fcb_row = pp.tile([1, V], BF16, tag="fcb")
            id_bf = pp.tile([128, 128], BF16, tag="ident")
            # 112 slack cols so M=128-padded lhsT slices [16k, 16k+128) stay
            # in-bounds (junk output partitions are ignored)
            hta = pp.tile([128, 240], BF16, tag="hta")
            htb = pp.tile([128, 240], BF16, tag="htb")
            hga = pp.tile([128, 512], BF16, tag="hga")
            hgb = pp.tile([128, 512], BF16, tag="hgb")
            ones_bf = pp.tile([1, 128], BF16, tag="onesb")

            nc.gpsimd.memset(ones_bf[:], 1.0)
            nc.gpsimd.memset(hta[:], 0.0)  # h_{-1} = 0 (+ slack cols stay 0)
            nc.gpsimd.memset(htb[:], 0.0)
            nc.gpsimd.memset(hga[:], 0.0)  # rows 16-127 stay 0 forever
            nc.gpsimd.memset(hgb[:], 0.0)
            nc.gpsimd.memset(ohwin[:], 0.0)  # slack cols stay 0

            nc.gpsimd.dma_start(whh_a[:], whh_a_d[:])
            nc.gpsimd.dma_start(whh_b[:], whh_b_d[:])
            nc.gpsimd.dma_start(wxh_a[:], wxh_a_d[:])
            nc.gpsimd.dma_start(wxh_b[:], wxh_b_d[:])
            nc.gpsimd.dma_start(oh[:], oh_d[:])
            nc.gpsimd.dma_start(fcw2_sb[:], fcw2_d[:])
            nc.gpsimd.dma_start(fcb_row[:], fcb_d[:])

            with tc.tile_pool(name="idtmp", bufs=1) as it_:
                id_f = it_.tile([128, 128], F32, tag="identf")
                make_identity(nc, id_f[:])
                nc.vector.tensor_copy(id_bf[:], id_f[:])

            hst_v = hst.rearrange("p (kb t) -> p kb t", t=t_steps)
            oh_w = oh.rearrange("p (t b) -> p t b", b=BS)
            ohwin_w = ohwin.rearrange("p (s b) -> p s b", b=BS)

            # ---- recurrence ----
            with (
                tc.tile_pool(name="psa", bufs=2, space="PSUM") as pap,
                tc.tile_pool(name="psb", bufs=2, space="PSUM") as pbp,
                tc.tile_pool(name="pst", bufs=2, space="PSUM") as ptp,
            ):
                def step(t_expr, parity, s=None, iv=None):
                    ht_cur = hta if parity == 0 else htb
                    ht_new = htb if parity == 0 else hta
                    h_grp = hga if parity == 0 else hgb
                    if s is None:
                        s = t_expr % UNROLL  # python-unrolled path
                    oh_t = ohwin_w[:, s, :]  # [128, 16] fixed address

                    pa = pap.tile([128, 512], F32, tag="pa", name="pa")
                    pb = pbp.tile([128, 64], F32, tag="pb", name="pb")

                    # Every matmul is (128,128)-mode (K=128, M=128): mode
                    # transitions drain the PE pipeline, so part-a pads its
                    # stationary to M=128 (output partitions 16-127 are junk
                    # and ignored). The weight port is the limiter
                    # (~54ns/128-col FWL load); part-a streams Whh through
                    # the moving port instead (1 load + N=512 stream per k).
                    # k=-1 is the vocab (onehot) contribution; chunks 4-7
                    # first (their h is ready earliest).
                    for k in (-1, 4, 5, 6, 7, 0, 1, 2, 3):
                        first, last = k == -1, k == 3
                        if k < 0:
                            lb = [wxh_b[:, 128 * jj : 128 * jj + 128] for jj in range(4)]
                            la = ohwin[:, 16 * s : 16 * s + 128]
                            ra = wxh_a[:, :]
                            rb = oh_t
                        else:
                            lb = [
                                whh_b[:, (jj * KCH + k) * 128 : (jj * KCH + k) * 128 + 128]
                                for jj in range(4)
                            ]
                            la = ht_cur[:, 16 * k : 16 * k + 128]
                            ra = whh_a[:, k * 512 : k * 512 + 512]
                            rb = ht_cur[:, 16 * k : 16 * k + 16]
                        nsplit = 2
                        for jj in range(nsplit):
                            nc.tensor.matmul(
                                pb[:, 16 * jj : 16 * jj + 16],
                                lb[jj],
                                rb,
                                start=(first and jj == 0), stop=last,
                                skip_group_check=True,
                            )
                        nc.tensor.matmul(
                            pa[:, :], la, ra,
                            start=first, stop=last,
                            skip_group_check=True,
                        )
                        for jj in range(nsplit, 4):
                            nc.tensor.matmul(
                                pb[:, 16 * jj : 16 * jj + 16],
                                lb[jj],
                                rb,
                                start=False, stop=last,
                                skip_group_check=True,
                            )
                        if first and s == UNROLL - 1 and iv is not None:
                            # prefetch next onehot window (single buffer; WAR
                            # on this step's vocab matmuls keeps semantics)
                            nc.vector.tensor_copy(
                                ohwin_w[:, 0:UNROLL, :],
                                oh_w[:, bass.DynSlice(iv + UNROLL, UNROLL), :],
                            )
                    # tanh: part-b lands hidden-major directly in ht chunks
                    # 4-7; part-a batch-major into h_grp rows 0-15 (2 halves
                    # so transposes can start after the first)
                    nc.scalar.activation(ht_new[:, 64:128], pb[:, :], AF.Tanh)
                    nc.scalar.activation(h_grp[0:16, 0:256], pa[0:16, 0:256], AF.Tanh)
                    nc.scalar.activation(h_grp[0:16, 256:512], pa[0:16, 256:512], AF.Tanh)
                    # part-a -> hidden-major chunks 0-3 via identity-rhs
                    # matmuls, still (128,128) mode: out[p,b] =
                    # sum_r h_grp[r, 128k+p] * I[r, b] = h[b, 128k+p]
                    pt = ptp.tile([128, 64], F32, tag="pt", name="pt")
                    for kk in range(4):
                        nc.tensor.matmul(
                            pt[:, 16 * kk : 16 * kk + 16],
                            h_grp[:, 128 * kk : 128 * kk + 128],
                            id_bf[:, 0:16],
                            start=(kk == 0), stop=(kk == 3),
                            skip_group_check=True,
                        )
                    nc.vector.tensor_copy(ht_new[:, 0:64], pt[:, :])
                    # append h_t to HsT (off critical path)
                    nc.vector.tensor_copy(
                        hst_v[:, :, bass.ts(t_expr, 1)],
                        ht_new.rearrange("p (kb one) -> p kb one", one=1)[:, 0:128, :],
                    )

                if t_steps <= 32:
                    for t in range(t_steps):
                        if t % UNROLL == 0:
                            nc.vector.tensor_copy(
                                ohwin_w[:, 0:UNROLL, :],
                                oh_w[:, t : t + UNROLL, :],
                            )
                        step(t, t % 2)
                else:
                    assert t_steps % UNROLL == 0
                    nc.vector.tensor_copy(
                        ohwin_w[:, 0:UNROLL, :], oh_w[:, 0:UNROLL, :]
                    )
                    with tc.For_i(
                        0, t_steps, UNROLL,
                        hint_engines=(mybir.EngineType.PE,),
                        staggered_reset=True,
                    ) as iv:
                        for s in range(UNROLL):
                            step(iv + s, s % 2, s=s, iv=iv)

            if hdbg_d is not None:
                nc.sync.dma_start(hdbg_d[:, :], hst[:, :])

            # ---- attention + fc, per batch ----
            with (
                tc.tile_pool(name="attn", bufs=1) as ap_,
                tc.tile_pool(name="attn2", bufs=2) as ap2,
                tc.tile_pool(name="psS", bufs=2, space="PSUM") as psS_p,
                tc.tile_pool(name="psF", bufs=2, space="PSUM") as psF_p,
                tc.tile_pool(name="ps2", bufs=2, space="PSUM") as ps2_p,
            ):
                n_tc = t_steps // 128  # t-chunks of 128
                for b in range(BS):
                    def hs(k, sl):  # HsT tile for (k-chunk, slice of t)
                        return hst_v[:, k * BS + b, sl]

                    p_sb = ap_.tile([128, n_tc * t_steps], BF16, tag="p_sb")
                    rinv = ap_.tile([128, n_tc], F32, tag="rinv")
                    g_sb = ap_.tile([128, n_tc * V], BF16, tag="g_sb")
                    # Fused pass per t-chunk: scores (N=512) and
                    # [out1 | G] (N=256, fcw halves fused) share each
                    # hs(k, c) stationary load. psF[:, 0:128] = Hs@fc_wh.T
                    # (+bias), psF[:, 128:256] = G chunk = Hs@fc_wc.T.
                    o1_sb = ap_.tile([128, n_tc * V], F32, tag="o1_sb")
                    for c in range(n_tc):
                        psS = psS_p.tile([128, t_steps], F32, tag="psS")
                        psF = psF_p.tile([128, 2 * V], F32, tag="psF", name="psF")
                        for k in range(KCH):
                            nc.tensor.matmul(
                                psS[:],
                                hs(k, slice(128 * c, 128 * c + 128)),
                                hs(k, slice(0, t_steps)),
                                start=(k == 0),
                                stop=(k == KCH - 1),
                            )
                            nc.tensor.matmul(
                                psF[:],
                                hs(k, slice(128 * c, 128 * c + 128)),
                                fcw2_sb[:, 256 * k : 256 * k + 256],
                                start=(k == 0),
                                stop=False,
                                skip_group_check=True,
                            )
                        nc.tensor.matmul(
                            psF[:, 0:V],
                            ones_bf[:],
                            fcb_row[:],
                            start=False,
                            stop=True,
                            skip_group_check=True,
                        )
                        rowsum = ap2.tile([128, 1], F32, tag="rowsum")
                        nc.scalar.activation(
                            p_sb[:, c * t_steps : (c + 1) * t_steps],
                            psS[:],
                            AF.Exp,
                            accum_out=rowsum[:],
                        )
                        nc.vector.reciprocal(rinv[:, c : c + 1], rowsum[:])
                        nc.vector.tensor_copy(
                            g_sb[:, c * V : (c + 1) * V], psF[:, V : 2 * V]
                        )
                        nc.vector.tensor_copy(
                            o1_sb[:, c * V : (c + 1) * V], psF[:, 0:V]
                        )
                    # out[t-chunk c] = psF[0:V] + rinv * (P @ G)
                    for c in range(n_tc):
                        ps2 = ps2_p.tile([128, V], F32, tag="ps2")
                        for i in range(n_tc):
                            # lhsT = P^T block (i,c) == P block, by symmetry
                            nc.tensor.matmul(
                                ps2[:],
                                p_sb[:, i * t_steps + 128 * c : i * t_steps + 128 * c + 128],
                                g_sb[:, i * V : (i + 1) * V],
                                start=(i == 0),
                                stop=(i == n_tc - 1),
                            )
                        o2 = ap2.tile([128, V], F32, tag="o2")
                        nc.vector.tensor_scalar_mul(o2[:], ps2[:], rinv[:, c : c + 1])
                        oo = ap2.tile([128, V], F32, tag="oo")
                        nc.vector.tensor_add(oo[:], o1_sb[:, c * V : (c + 1) * V], o2[:])
                        nc.sync.dma_start(out_d[b, 128 * c : 128 * c + 128, :], oo[:])

    nc.compile()
    return nc


def _prep_core_inputs(inputs, core, t_steps=T):
    x = np.asarray(inputs["x"])[core * BS : (core + 1) * BS, :t_steps]
    wxhb = (
        np.asarray(inputs["Wxh"]).astype(np.float32)
        + np.asarray(inputs["bh"]).astype(np.float32)[None, :]
    )
    whh = np.asarray(inputs["Whh"]).astype(np.float32)
    w4 = whh.reshape(KCH, 128, KCH, 128)
    # whh_a[p, (k, g, j)] = Whh[128k+p, 128g+j], g < 4
    whh_a = np.ascontiguousarray(w4[:, :, :4, :].transpose(1, 0, 2, 3)).reshape(128, -1)
    # whh_b[p, (jj, k, j)] = Whh[128k+p, 512 + 128jj + j]
    whh_b = np.ascontiguousarray(w4[:, :, 4:, :].transpose(1, 2, 0, 3)).reshape(128, -1)
    # onehot: oh[v, 16t + b] = (x[b, t] == v); one zero window of padding
    oh = np.zeros((V, (t_steps + UNROLL) * BS), dtype=BFNP)
    oh[x.T.reshape(-1), np.arange(t_steps * BS)] = 1
    return {
        "whh_a": whh_a.astype(BFNP),
        "whh_b": whh_b.astype(BFNP),
        "wxh_a": np.ascontiguousarray(wxhb[:, :512]).astype(BFNP),
        "wxh_b": np.ascontiguousarray(wxhb[:, 512:]).astype(BFNP),
        "oh": oh,
        # fcw2[p, k*256 + half*128 + v] = fc_w[v, half*1024 + 128k + p]
        "fcw2": np.ascontiguousarray(
            np.asarray(inputs["fc_w"])
            .astype(np.float32)
            .T.reshape(2, KCH, 128, V)
            .transpose(2, 1, 0, 3)
        ).reshape(128, 16 * V).astype(BFNP),
        "fcb": np.asarray(inputs["fc_b"]).astype(np.float32).reshape(1, V).astype(BFNP),
    }


def kernel(x, Wxh, Whh, bh, fc_w, fc_b, t_steps=T, trace=False):
    inputs = dict(x=x, Wxh=Wxh, Whh=Whh, bh=bh, fc_w=fc_w, fc_b=fc_b)
    nc = build_nc(t_steps)
    in_maps = [_prep_core_inputs(inputs, c, t_steps) for c in range(NCORES)]
    res = run_bass_kernel_spmd(nc, in_maps, core_ids=list(range(NCORES)), trace=trace)
    out = np.concatenate([r["out"] for r in res.results], axis=0)
    if trace:
        print(f"HW exec time: {res.exec_time_ns} ns", file=sys.stderr)
    return out
